# revision 18
# baseline (speedup 1.0000x reference)
"""GCN layer kernel for Trainium2, 8 NeuronCores.

Math (reference):
    a      = adj + I                      (self loops)
    deg_i  = sum_j a[i, j]
    d      = deg ** -0.5                  (deg >= 1 always, no inf case)
    out    = relu( (d[:,None] * a * d[None,:]) @ (x @ W + b) )

Distribution: 1-D node partition over 8 cores. a is symmetric, so core c's
row block a[rows_c, :] equals the column slab a[:, rows_c] transposed. Each
core receives the column slab `aslab = a[:, rows_c]` (50 MB fp32) and
streams it from HBM exactly once:
  - degrees of its own rows via a ones-vector matmul (partition reduction),
  - the slab is cast to fp8e4 (exact for the 0/1 entries) into a persistent
    12.5 MB SBUF cache.
A 5 KB AllGather shares the per-core degree vectors. The propagate matmul
then runs entirely from the SBUF fp8 cache (no second HBM pass):
    yT[f, i] = sum_j H'[j, f] * aslab[j, i],  H' = d (.) h,  h = x@W+b
with H' split hi/lo into bf16 pairs packed side by side in the stationary
operand (f32-level accuracy at no extra streaming cost; fp8 moving x bf16
stationary mixed-dtype matmul is legal on TRN2). Output is produced
feature-major [64, rows] and transposed on the host.
"""

import numpy as np

import concourse.bacc as bacc
import concourse.tile as tile
from concourse import mybir
from concourse.bass_utils import run_bass_kernel_spmd

F32 = mybir.dt.float32
BF16 = mybir.dt.bfloat16
FP8 = mybir.dt.float8e4

N = 10000
D = 64
N_CORES = 8
P = 125  # partition tile size (N % P == 0)


def _chunks(total, step=512):
    return [(s, min(step, total - s)) for s in range(0, total, step)]


def _emit_body(nc, tc, aps, cfg, rep, stub_collective=False, variant="full"):
    n, d, n_cores, p = cfg
    rows = n // n_cores
    nt = n // p
    ch = _chunks(rows)
    da = d + 1
    xch = 5 * p if n % (5 * p) == 0 else rows
    nxch = n // xch
    aslab, xat, wb, outT = aps
    r = f"r{rep}"

    with (
        tc.tile_pool(name=f"const{r}", bufs=1) as constp,
        tc.tile_pool(name=f"xatp{r}", bufs=2) as xatp,
        tc.tile_pool(name=f"hpool{r}", bufs=nt) as hpool,
        tc.tile_pool(name=f"stage{r}", bufs=3) as stagep,
        tc.tile_pool(name=f"slab8{r}", bufs=nt // 2) as slab8p,
        tc.tile_pool(name=f"h8p{r}", bufs=nt) as h8p,
        tc.tile_pool(name=f"small{r}", bufs=1) as smallp,
        tc.tile_pool(name=f"rowvec{r}", bufs=2) as rowvecp,
        tc.tile_pool(name=f"tmpp{r}", bufs=3) as tmpp,
        tc.tile_pool(name=f"finp{r}", bufs=1) as finp,
        tc.tile_pool(name=f"hps{r}", bufs=2, space="PSUM") as hps,
        tc.tile_pool(name=f"cps{r}", bufs=1, space="PSUM") as cps,
        tc.tile_pool(name=f"yps{r}", bufs=1, space="PSUM") as yps,
        tc.tile_pool(name=f"dram{r}", bufs=1, space="DRAM") as dramp,
    ):
        # ---- constants ----
        wb_sb = constp.tile([da, d], F32, name=f"wb_sb{r}")
        nc.sync.dma_start(out=wb_sb[:], in_=wb[:, :])
        # bf16 ones: stationary for the degree matmul over the fp8 slab
        # (fp32 moving operands stream at 1/4 rate - avoid fp32 matmuls)
        ones_p = constp.tile([p, 1], BF16, name=f"ones_p{r}")
        nc.gpsimd.memset(ones_p[:], 1.0)
        ones_m = constp.tile([1, d], F32, name=f"ones_m{r}")
        nc.gpsimd.memset(ones_m[:], 1.0)

        # ---- stream the slab: degree matmuls + fp8 cast ----
        deg_ps = [
            cps.tile([1, nn], F32, tag=f"c{i}", name=f"degps{i}{r}")
            for i, (s, nn) in enumerate(ch)
        ]
        slab_tiles = []  # per-PAIR fp8 tiles [p, 2*rows]
        for q in range(nt // 2):
            st = stagep.tile([p, 2 * rows], F32, tag="stage", name=f"st{q}{r}")
            src = aslab[2 * q * p : (2 * q + 2) * p, :].rearrange(
                "(a q2) w -> q2 a w", a=2
            )
            nc.sync.dma_start(
                out=st[:].rearrange("q2 (a w) -> q2 a w", a=2), in_=src
            )
            if variant == "dmaonly":
                continue
            s8 = slab8p.tile([p, 2 * rows], FP8, tag="s8", name=f"s8_{q}{r}")
            # split the casts across three engines to stay under the DMA shadow
            # (DVE is ~2x faster than ACT/GpSimd for this copy - give it half)
            if q % 4 in (0, 2):
                nc.vector.tensor_copy(s8[:], st[:])
            elif q % 4 == 1:
                nc.scalar.copy(s8[:], st[:])
            else:
                nc.gpsimd.tensor_copy(s8[:], st[:])
            slab_tiles.append(s8)
            if variant == "nodeg":
                continue
            # degrees from the fp8 cache (exact for 0/1): full-rate streaming
            for a in range(2):
                t = 2 * q + a
                for i, (s, nn) in enumerate(ch):
                    nc.tensor.matmul(
                        deg_ps[i][:],
                        lhsT=ones_p[:],
                        rhs=s8[:, a * rows + s : a * rows + s + nn],
                        start=(t == 0),
                        stop=(t == nt - 1),
                    )

        if variant in ("dmaonly", "nodeg"):
            outT_sb1 = smallp.tile([d, rows], F32, name=f"outT_sb1{r}")
            nc.gpsimd.memset(outT_sb1[:], 0.0)
            nc.sync.dma_start(out=outT[:, :], in_=outT_sb1[:])
            return

        # ---- h = [x|1] @ [W;b] ----
        # emitted AFTER the slab stream: its DMA fills the collective-wait
        # gap and its matmuls keep PE warm (HAM) into the propagate phase
        h_tiles = []
        for ci in range(nxch):
            xat_c = xatp.tile([da, xch], F32, tag="xat_c", name=f"xat{ci}{r}")
            nc.sync.dma_start(out=xat_c[:], in_=xat[:, ci * xch : (ci + 1) * xch])
            for tj in range(xch // p):
                t = ci * (xch // p) + tj
                hp = hps.tile([p, d], F32, tag="hp", name=f"hp{t}{r}")
                nc.tensor.matmul(
                    hp[:],
                    lhsT=xat_c[:, tj * p : (tj + 1) * p],
                    rhs=wb_sb[:],
                    start=True,
                    stop=True,
                )
                h_sb = hpool.tile([p, d], F32, tag="h", name=f"h{t}{r}")
                nc.vector.tensor_copy(h_sb[:], hp[:])
                h_tiles.append(h_sb)

        # ---- degrees of my rows -> AllGather -> d scalers ----
        deg_row = rowvecp.tile([1, rows], F32, tag="rv", name=f"deg_row{r}")
        for i, (s, nn) in enumerate(ch):
            nc.vector.tensor_copy(deg_row[:, s : s + nn], deg_ps[i][:])

        deg_in = dramp.tile([1, rows], F32, name=f"deg_in{r}")
        deg_out = dramp.tile([1, n], F32, name=f"deg_out{r}")
        nc.sync.dma_start(out=deg_in[:], in_=deg_row[:])
        if stub_collective:
            # single-core timeline model: replicate instead of AllGather
            for c in range(n_cores):
                nc.sync.dma_start(
                    out=deg_out[0:1, c * rows : (c + 1) * rows], in_=deg_in[:]
                )
        else:
            nc.gpsimd.collective_compute(
                "AllGather",
                mybir.AluOpType.bypass,
                replica_groups=[list(range(n_cores))],
                ins=[deg_in.opt()],
                outs=[deg_out.opt()],
            )

        # d_all[q, t] = deg(node t*p + q), strided load of the gathered vec
        dall_raw = smallp.tile([p, nt], F32, name=f"dall_raw{r}")
        nc.sync.dma_start(
            out=dall_raw[:],
            in_=deg_out[0, :].rearrange("(t q) -> q t", q=p),
        )

        def rsqrt(dst, src, tmp_tag, shape):
            # dst = src ** -0.5 via ACT sqrt + DVE reciprocal + one Newton step
            sq = tmpp.tile(shape, F32, tag=tmp_tag, name=f"sq_{tmp_tag}{r}", bufs=2)
            nc.scalar.sqrt(sq[:], src[:])
            nc.vector.reciprocal(dst[:], sq[:])
            t1 = tmpp.tile(shape, F32, tag=tmp_tag, name=f"t1_{tmp_tag}{r}", bufs=2)
            nc.vector.tensor_mul(t1[:], dst[:], dst[:])
            nc.vector.tensor_mul(t1[:], t1[:], src[:])
            nc.vector.tensor_scalar(
                out=t1[:],
                in0=t1[:],
                scalar1=-0.5,
                scalar2=1.5,
                op0=mybir.AluOpType.mult,
                op1=mybir.AluOpType.add,
            )
            nc.vector.tensor_mul(dst[:], dst[:], t1[:])

        dall = smallp.tile([p, nt], F32, name=f"dall{r}")
        rsqrt(dall, dall_raw, "dn", [p, nt])
        # d of my rows: rsqrt in a partition-parallel [p, rows/p] layout
        # (single-partition [1, rows] DVE math costs ~25us), then bounce
        # through DRAM to the [1, rows] free-layout the outer product needs
        ntl = rows // p
        degl = smallp.tile([p, ntl], F32, name=f"degl{r}")
        nc.sync.dma_start(
            out=degl[:], in_=deg_in[0, :].rearrange("(t q) -> q t", q=p)
        )
        drl = smallp.tile([p, ntl], F32, name=f"drl{r}")
        rsqrt(drl, degl, "dl", [p, ntl])
        d_loc = dramp.tile([1, rows], F32, name=f"d_loc{r}")
        nc.sync.dma_start(
            out=d_loc[0, :].rearrange("(t q) -> q t", q=p), in_=drl[:]
        )
        drow = rowvecp.tile([1, rows], F32, tag="rv", name=f"drow{r}")
        nc.sync.dma_start(out=drow[:], in_=d_loc[:])

        # dbc[f, i] = drow[i] broadcast over features, via ones outer product
        dbc_sb = smallp.tile([d, rows], F32, name=f"dbc_sb{r}")
        for i, (s, nn) in enumerate(ch):
            dps = cps.tile([d, nn], F32, tag=f"c{i}", name=f"dbc{i}{r}")
            nc.tensor.matmul(
                dps[:],
                lhsT=ones_m[:],
                rhs=drow[:, s : s + nn],
                start=True,
                stop=True,
            )
            nc.vector.tensor_copy(dbc_sb[:, s : s + nn], dps[:])

        # ---- H' = d (.) h, split hi/lo bf16, packed [p, 2d] ----
        h8_tiles = []
        for t in range(nt):
            tmpf = tmpp.tile([p, d], F32, tag="hprime", name=f"hpr{t}{r}")
            nc.scalar.mul(tmpf[:], h_tiles[t][:], dall[:, t : t + 1])
            h8 = h8p.tile([p, 2 * d], BF16, tag="h8", name=f"h8_{t}{r}")
            nc.vector.tensor_copy(h8[:, 0:d], tmpf[:])
            nc.vector.tensor_sub(h8[:, d : 2 * d], tmpf[:], h8[:, 0:d])
            h8_tiles.append(h8)

        # ---- propagate from the SBUF fp8 cache ----
        if variant == "nopc":
            # timing variant: skip the propagate matmuls + finalize
            outT_sb0 = smallp.tile([d, rows], F32, name=f"outT_sb0{r}")
            nc.vector.tensor_copy(outT_sb0[:], dbc_sb[:])
            nc.sync.dma_start(out=outT[:, :], in_=outT_sb0[:])
            return
        y_ps = [
            yps.tile([2 * d, nn], F32, tag=f"y{i}", name=f"yps{i}{r}")
            for i, (s, nn) in enumerate(ch)
        ]
        for t in range(nt):
            q, a = divmod(t, 2)
            for i, (s, nn) in enumerate(ch):
                nc.tensor.matmul(
                    y_ps[i][:],
                    lhsT=h8_tiles[t][:],
                    rhs=slab_tiles[q][:, a * rows + s : a * rows + s + nn],
                    start=(t == 0),
                    stop=(t == nt - 1),
                )

        # ---- finalize: y = relu(dbc * (y_hi + y_lo)), write outT ----
        outT_sb = smallp.tile([d, rows], F32, name=f"outT_sb{r}")
        for i, (s, nn) in enumerate(ch):
            ylo = finp.tile([d, nn], F32, tag="ylo", name=f"ylo{i}{r}")
            # cross-quadrant DVE move: read psum parts [d:2d), write [0:d)
            nc.vector.tensor_copy(ylo[:], y_ps[i][d : 2 * d, :])
            ysum = finp.tile([d, nn], F32, tag="ysum", name=f"ysum{i}{r}")
            nc.vector.tensor_add(ysum[:], y_ps[i][0:d, :], ylo[:])
            yfin = finp.tile([d, nn], F32, tag="yfin", name=f"yfin{i}{r}")
            nc.vector.tensor_mul(yfin[:], ysum[:], dbc_sb[:, s : s + nn])
            nc.scalar.activation(
                outT_sb[:, s : s + nn],
                yfin[:],
                mybir.ActivationFunctionType.Relu,
            )
        nc.sync.dma_start(out=outT[:, :], in_=outT_sb[:])


def build_nc(n=N, d=D, n_cores=N_CORES, p=P, repeats=1, stub_collective=False, variant="full"):
    rows = n // n_cores
    da = d + 1

    nc = bacc.Bacc(
        "TRN2",
        target_bir_lowering=False,
        debug=False,
        enable_asserts=False,
        num_devices=1 if stub_collective else n_cores,
    )
    aslab = nc.dram_tensor("aslab", [n, rows], F32, kind="ExternalInput").ap()
    xat = nc.dram_tensor("xat", [da, n], F32, kind="ExternalInput").ap()
    wb = nc.dram_tensor("wb", [da, d], F32, kind="ExternalInput").ap()
    outT = nc.dram_tensor("outT", [d, rows], F32, kind="ExternalOutput").ap()
    aps = (aslab, xat, wb, outT)
    cfg = (n, d, n_cores, p)

    with tile.TileContext(nc) as tc:
        for rep in range(repeats):
            if rep:
                tc.strict_bb_all_engine_barrier()
            _emit_body(nc, tc, aps, cfg, rep, stub_collective=stub_collective, variant=variant)

    nc.compile()
    return nc


_NC_CACHE = {}


def _get_nc(key=(N, D, N_CORES, P)):
    if key not in _NC_CACHE:
        _NC_CACHE[key] = build_nc(*key)
    return _NC_CACHE[key]


def host_prep(x, adj_matrix, W, b, n=N, n_cores=N_CORES):
    x = np.asarray(x, np.float32)
    adj = np.asarray(adj_matrix, np.float32)
    W = np.asarray(W, np.float32)
    b = np.asarray(b, np.float32)
    rows = n // n_cores
    xat = np.concatenate([x.T, np.ones((1, n), np.float32)], axis=0)
    xat = np.ascontiguousarray(xat)
    wb = np.ascontiguousarray(np.concatenate([W, b[None, :]], axis=0))
    in_maps = []
    idx = np.arange(rows)
    for c in range(n_cores):
        lo = c * rows
        slab = np.ascontiguousarray(adj[:, lo : lo + rows])
        slab[lo + idx, idx] += 1.0  # self loops (a = adj + I)
        in_maps.append({"aslab": slab, "xat": xat, "wb": wb})
    return in_maps


def gather(results, n_cores=N_CORES):
    return np.ascontiguousarray(
        np.concatenate([np.asarray(r["outT"]).T for r in results[:n_cores]], axis=0)
    ).astype(np.float32)


def kernel(x, adj_matrix, W, b):
    nc = _get_nc()
    in_maps = host_prep(x, adj_matrix, W, b)
    res = run_bass_kernel_spmd(nc, in_maps, core_ids=list(range(N_CORES)))
    return gather(res.results)


if __name__ == "__main__":
    rng = np.random.default_rng(0)
    x = rng.standard_normal((N, D)).astype(np.float32)
    u = rng.random((N, N)).astype(np.float32)
    adj = (u < 0.01).astype(np.float32)
    adj = np.maximum(adj, adj.T) * (1.0 - np.eye(N, dtype=np.float32))
    W = rng.standard_normal((D, D)).astype(np.float32) / 8.0
    b = np.zeros(D, np.float32)
    out = kernel(x, adj, W, b)
    print(out.shape, out.dtype, float(np.abs(out).max()))


# revision 21
# speedup vs baseline: 1.1197x; 1.1197x over previous
"""GCN layer kernel for Trainium2, 8 NeuronCores.

Math (reference):
    a      = adj + I                      (self loops)
    deg_i  = sum_j a[i, j]
    d      = deg ** -0.5                  (deg >= 1 always, no inf case)
    out    = relu( (d[:,None] * a * d[None,:]) @ (x @ W + b) )

Distribution: 1-D node partition over 8 cores. a is symmetric, so core c's
row block a[rows_c, :] equals the column slab a[:, rows_c] transposed. Each
core receives the column slab `aslab = a[:, rows_c]` (50 MB fp32) and
streams it from HBM exactly once:
  - degrees of its own rows via a ones-vector matmul (partition reduction),
  - the slab is cast to fp8e4 (exact for the 0/1 entries) into a persistent
    12.5 MB SBUF cache.
A 5 KB AllGather shares the per-core degree vectors. The propagate matmul
then runs entirely from the SBUF fp8 cache (no second HBM pass):
    yT[f, i] = sum_j H'[j, f] * aslab[j, i],  H' = d (.) h,  h = x@W+b
with H' split hi/lo into bf16 pairs packed side by side in the stationary
operand (f32-level accuracy at no extra streaming cost; fp8 moving x bf16
stationary mixed-dtype matmul is legal on TRN2). Output is produced
feature-major [64, rows] and transposed on the host.
"""

import numpy as np

import concourse.bacc as bacc
import concourse.tile as tile
from concourse import mybir
from concourse.bass_utils import run_bass_kernel_spmd

F32 = mybir.dt.float32
BF16 = mybir.dt.bfloat16
FP8 = mybir.dt.float8e4

N = 10000
D = 64
N_CORES = 8
P = 125  # partition tile size (N % P == 0)


def _chunks(total, step=512):
    return [(s, min(step, total - s)) for s in range(0, total, step)]


def _emit_body(nc, tc, aps, cfg, rep, stub_collective=False, variant="full", dr_mode=False):
    n, d, n_cores, p = cfg
    rows = n // n_cores
    nt = n // p
    ch = _chunks(rows)
    da = d + 1
    xch = 5 * p if n % (5 * p) == 0 else rows
    nxch = n // xch
    aslab, xat, wb, outT = aps
    r = f"r{rep}"

    with (
        tc.tile_pool(name=f"const{r}", bufs=1) as constp,
        tc.tile_pool(name=f"xatp{r}", bufs=2) as xatp,
        tc.tile_pool(name=f"hpool{r}", bufs=nt) as hpool,
        tc.tile_pool(name=f"stage{r}", bufs=3) as stagep,
        tc.tile_pool(name=f"slab8{r}", bufs=nt // 2) as slab8p,
        tc.tile_pool(name=f"h8p{r}", bufs=nt) as h8p,
        tc.tile_pool(name=f"small{r}", bufs=1) as smallp,
        tc.tile_pool(name=f"rowvec{r}", bufs=2) as rowvecp,
        tc.tile_pool(name=f"tmpp{r}", bufs=3) as tmpp,
        tc.tile_pool(name=f"finp{r}", bufs=1) as finp,
        tc.tile_pool(name=f"hps{r}", bufs=2, space="PSUM") as hps,
        tc.tile_pool(name=f"cps{r}", bufs=1, space="PSUM") as cps,
        tc.tile_pool(name=f"yps{r}", bufs=1, space="PSUM") as yps,
        tc.tile_pool(name=f"dram{r}", bufs=1, space="DRAM") as dramp,
    ):
        # ---- constants ----
        wb_sb = constp.tile([da, d], F32, name=f"wb_sb{r}")
        nc.sync.dma_start(out=wb_sb[:], in_=wb[:, :])
        # bf16 ones: stationary for the degree matmul over the fp8 slab
        # (fp32 moving operands stream at 1/4 rate - avoid fp32 matmuls)
        ones_p = constp.tile([p, 1], BF16, name=f"ones_p{r}")
        nc.gpsimd.memset(ones_p[:], 1.0)
        ones_m = constp.tile([1, d], F32, name=f"ones_m{r}")
        nc.gpsimd.memset(ones_m[:], 1.0)

        # ---- stream the slab: degree matmuls + fp8 cast ----
        deg_ps = [
            cps.tile([1, nn], F32, tag=f"c{i}", name=f"degps{i}{r}")
            for i, (s, nn) in enumerate(ch)
        ]
        slab_tiles = []  # per-PAIR fp8 tiles [p, 2*rows]
        for q in range(nt // 2):
            st = stagep.tile([p, 2 * rows], F32, tag="stage", name=f"st{q}{r}")
            src = aslab[2 * q * p : (2 * q + 2) * p, :].rearrange(
                "(a q2) w -> q2 a w", a=2
            )
            nc.sync.dma_start(
                out=st[:].rearrange("q2 (a w) -> q2 a w", a=2), in_=src
            )
            if variant == "dmaonly":
                continue
            s8 = slab8p.tile([p, 2 * rows], FP8, tag="s8", name=f"s8_{q}{r}")
            # split the casts across three engines to stay under the DMA shadow
            # (DVE is ~2x faster than ACT/GpSimd for this copy - give it half)
            if q % 4 in (0, 2):
                nc.vector.tensor_copy(s8[:], st[:])
            elif q % 4 == 1:
                nc.scalar.copy(s8[:], st[:])
            else:
                nc.gpsimd.tensor_copy(s8[:], st[:])
            slab_tiles.append(s8)
            if variant == "nodeg":
                continue
            # degrees from the fp8 cache (exact for 0/1): full-rate streaming
            for a in range(2):
                t = 2 * q + a
                for i, (s, nn) in enumerate(ch):
                    nc.tensor.matmul(
                        deg_ps[i][:],
                        lhsT=ones_p[:],
                        rhs=s8[:, a * rows + s : a * rows + s + nn],
                        start=(t == 0),
                        stop=(t == nt - 1),
                    )

        if variant in ("dmaonly", "nodeg"):
            outT_sb1 = smallp.tile([d, rows], F32, name=f"outT_sb1{r}")
            nc.gpsimd.memset(outT_sb1[:], 0.0)
            nc.sync.dma_start(out=outT[:, :], in_=outT_sb1[:])
            return

        # ---- h = [x|1] @ [W;b] ----
        # emitted AFTER the slab stream: its DMA fills the collective-wait
        # gap and its matmuls keep PE warm (HAM) into the propagate phase
        h_tiles = []
        for ci in range(nxch):
            xat_c = xatp.tile([da, xch], F32, tag="xat_c", name=f"xat{ci}{r}")
            nc.sync.dma_start(out=xat_c[:], in_=xat[:, ci * xch : (ci + 1) * xch])
            for tj in range(xch // p):
                t = ci * (xch // p) + tj
                hp = hps.tile([p, d], F32, tag="hp", name=f"hp{t}{r}")
                nc.tensor.matmul(
                    hp[:],
                    lhsT=xat_c[:, tj * p : (tj + 1) * p],
                    rhs=wb_sb[:],
                    start=True,
                    stop=True,
                )
                h_sb = hpool.tile([p, d], F32, tag="h", name=f"h{t}{r}")
                nc.vector.tensor_copy(h_sb[:], hp[:])
                h_tiles.append(h_sb)

        # ---- degrees of my rows -> AllGather -> d scalers ----
        deg_row = rowvecp.tile([1, rows], F32, tag="rv", name=f"deg_row{r}")
        for i, (s, nn) in enumerate(ch):
            nc.vector.tensor_copy(deg_row[:, s : s + nn], deg_ps[i][:])

        deg_in = dramp.tile([1, rows], F32, name=f"deg_in{r}")
        deg_out = dramp.tile([1, n], F32, name=f"deg_out{r}")
        nc.sync.dma_start(out=deg_in[:], in_=deg_row[:])
        if stub_collective:
            # single-core timeline model: replicate instead of AllGather
            for c in range(n_cores):
                nc.sync.dma_start(
                    out=deg_out[0:1, c * rows : (c + 1) * rows], in_=deg_in[:]
                )
        else:
            nc.gpsimd.collective_compute(
                "AllGather",
                mybir.AluOpType.bypass,
                replica_groups=[list(range(n_cores))],
                ins=[deg_in.opt()],
                outs=[deg_out.opt()],
            )

        # d_all[q, t] = deg(node t*p + q), strided load of the gathered vec
        dall_raw = smallp.tile([p, nt], F32, name=f"dall_raw{r}")
        nc.sync.dma_start(
            out=dall_raw[:],
            in_=deg_out[0, :].rearrange("(t q) -> q t", q=p),
        )

        def rsqrt(dst, src, tmp_tag, shape):
            # dst = src ** -0.5 via ACT sqrt + DVE reciprocal + one Newton step
            sq = tmpp.tile(shape, F32, tag=tmp_tag, name=f"sq_{tmp_tag}{r}", bufs=2)
            nc.scalar.sqrt(sq[:], src[:])
            nc.vector.reciprocal(dst[:], sq[:])
            t1 = tmpp.tile(shape, F32, tag=tmp_tag, name=f"t1_{tmp_tag}{r}", bufs=2)
            nc.vector.tensor_mul(t1[:], dst[:], dst[:])
            nc.vector.tensor_mul(t1[:], t1[:], src[:])
            nc.vector.tensor_scalar(
                out=t1[:],
                in0=t1[:],
                scalar1=-0.5,
                scalar2=1.5,
                op0=mybir.AluOpType.mult,
                op1=mybir.AluOpType.add,
            )
            nc.vector.tensor_mul(dst[:], dst[:], t1[:])

        dall = smallp.tile([p, nt], F32, name=f"dall{r}")
        rsqrt(dall, dall_raw, "dn", [p, nt])
        # d of my rows: rsqrt in a partition-parallel [p, rows/p] layout
        # (single-partition [1, rows] DVE math costs ~25us), then bounce
        # through DRAM to the [1, rows] free-layout the outer product needs
        ntl = rows // p
        degl = smallp.tile([p, ntl], F32, name=f"degl{r}")
        nc.sync.dma_start(
            out=degl[:], in_=deg_in[0, :].rearrange("(t q) -> q t", q=p)
        )
        drl = smallp.tile([p, ntl], F32, name=f"drl{r}")
        rsqrt(drl, degl, "dl", [p, ntl])
        d_loc = dramp.tile([1, rows], F32, name=f"d_loc{r}")
        nc.sync.dma_start(
            out=d_loc[0, :].rearrange("(t q) -> q t", q=p), in_=drl[:]
        )
        drow = rowvecp.tile([1, rows], F32, tag="rv", name=f"drow{r}")
        nc.sync.dma_start(out=drow[:], in_=d_loc[:])

        # dbc[f, i] = drow[i] broadcast over features, via ones outer product
        dbc_sb = smallp.tile([d, rows], F32, name=f"dbc_sb{r}")
        for i, (s, nn) in enumerate(ch):
            dps = cps.tile([d, nn], F32, tag=f"c{i}", name=f"dbc{i}{r}")
            nc.tensor.matmul(
                dps[:],
                lhsT=ones_m[:],
                rhs=drow[:, s : s + nn],
                start=True,
                stop=True,
            )
            nc.vector.tensor_copy(dbc_sb[:, s : s + nn], dps[:])

        # ---- H' = d (.) h, split hi/lo, packed [p, 2d] per k-tile ----
        # dr_mode: fp8 pairs [p, 2, 2d] for DoubleRow (2 fp8 weights/cell)
        h8_tiles = []
        hdt = FP8 if dr_mode else BF16
        if dr_mode:
            h8_pairs = [
                h8p.tile([p, 4 * d], FP8, tag="h8", name=f"h8p_{u}{r}", bufs=nt // 2)
                for u in range(nt // 2)
            ]
        for t in range(nt):
            tmpf = tmpp.tile([p, d], F32, tag="hprime", name=f"hpr{t}{r}")
            nc.scalar.mul(tmpf[:], h_tiles[t][:], dall[:, t : t + 1])
            if dr_mode:
                u, aa = divmod(t, 2)
                h8 = h8_pairs[u]
                o = aa * 2 * d
            else:
                h8 = h8p.tile([p, 2 * d], hdt, tag="h8", name=f"h8_{t}{r}")
                o = 0
            nc.vector.tensor_copy(h8[:, o : o + d], tmpf[:])
            nc.vector.tensor_sub(h8[:, o + d : o + 2 * d], tmpf[:], h8[:, o : o + d])
            if not dr_mode:
                h8_tiles.append(h8)

        # ---- propagate from the SBUF fp8 cache ----
        if variant == "nopc":
            # timing variant: skip the propagate matmuls + finalize
            outT_sb0 = smallp.tile([d, rows], F32, name=f"outT_sb0{r}")
            nc.vector.tensor_copy(outT_sb0[:], dbc_sb[:])
            nc.sync.dma_start(out=outT[:, :], in_=outT_sb0[:])
            return
        y_ps = [
            yps.tile([2 * d, nn], F32, tag=f"y{i}", name=f"yps{i}{r}")
            for i, (s, nn) in enumerate(ch)
        ]
        if dr_mode:
            for u in range(nt // 2):
                lhsT = h8_pairs[u][:].rearrange("q (a m) -> q a m", a=2)
                rhs3 = slab_tiles[u][:].rearrange("q (a w) -> q a w", a=2)
                for i, (s, nn) in enumerate(ch):
                    nc.tensor.matmul(
                        y_ps[i][:],
                        lhsT=lhsT,
                        rhs=rhs3[:, :, s : s + nn],
                        start=(u == 0),
                        stop=(u == nt // 2 - 1),
                        perf_mode=mybir.MatmulPerfMode.DoubleRow,
                    )
        else:
            for t in range(nt):
                q, a = divmod(t, 2)
                for i, (s, nn) in enumerate(ch):
                    nc.tensor.matmul(
                        y_ps[i][:],
                        lhsT=h8_tiles[t][:],
                        rhs=slab_tiles[q][:, a * rows + s : a * rows + s + nn],
                        start=(t == 0),
                        stop=(t == nt - 1),
                    )

        # ---- finalize: y = relu(dbc * (y_hi + y_lo)), write outT ----
        outT_sb = smallp.tile([d, rows], F32, name=f"outT_sb{r}")
        for i, (s, nn) in enumerate(ch):
            ylo = finp.tile([d, nn], F32, tag="ylo", name=f"ylo{i}{r}")
            # cross-quadrant DVE move: read psum parts [d:2d), write [0:d)
            nc.vector.tensor_copy(ylo[:], y_ps[i][d : 2 * d, :])
            ysum = finp.tile([d, nn], F32, tag="ysum", name=f"ysum{i}{r}")
            nc.vector.tensor_add(ysum[:], y_ps[i][0:d, :], ylo[:])
            yfin = finp.tile([d, nn], F32, tag="yfin", name=f"yfin{i}{r}")
            nc.vector.tensor_mul(yfin[:], ysum[:], dbc_sb[:, s : s + nn])
            nc.scalar.activation(
                outT_sb[:, s : s + nn],
                yfin[:],
                mybir.ActivationFunctionType.Relu,
            )
        nc.sync.dma_start(out=outT[:, :], in_=outT_sb[:])


def build_nc(n=N, d=D, n_cores=N_CORES, p=P, repeats=1, stub_collective=False, variant="full", barriers=1, dr_mode=False):
    rows = n // n_cores
    da = d + 1

    nc = bacc.Bacc(
        "TRN2",
        target_bir_lowering=False,
        debug=False,
        enable_asserts=False,
        num_devices=1 if stub_collective else n_cores,
    )
    aslab = nc.dram_tensor("aslab", [n, rows], F32, kind="ExternalInput").ap()
    xat = nc.dram_tensor("xat", [da, n], F32, kind="ExternalInput").ap()
    wb = nc.dram_tensor("wb", [da, d], F32, kind="ExternalInput").ap()
    outT = nc.dram_tensor("outT", [d, rows], F32, kind="ExternalOutput").ap()
    aps = (aslab, xat, wb, outT)
    cfg = (n, d, n_cores, p)

    with tile.TileContext(nc) as tc:
        for rep in range(repeats):
            if rep:
                for _ in range(barriers):
                    tc.strict_bb_all_engine_barrier()
            _emit_body(nc, tc, aps, cfg, rep, stub_collective=stub_collective, variant=variant, dr_mode=dr_mode)

    nc.compile()
    return nc


_NC_CACHE = {}


def _get_nc(key=(N, D, N_CORES, P)):
    if key not in _NC_CACHE:
        _NC_CACHE[key] = build_nc(*key)
    return _NC_CACHE[key]


def host_prep(x, adj_matrix, W, b, n=N, n_cores=N_CORES):
    x = np.asarray(x, np.float32)
    adj = np.asarray(adj_matrix, np.float32)
    W = np.asarray(W, np.float32)
    b = np.asarray(b, np.float32)
    rows = n // n_cores
    xat = np.concatenate([x.T, np.ones((1, n), np.float32)], axis=0)
    xat = np.ascontiguousarray(xat)
    wb = np.ascontiguousarray(np.concatenate([W, b[None, :]], axis=0))
    in_maps = []
    idx = np.arange(rows)
    for c in range(n_cores):
        lo = c * rows
        slab = np.ascontiguousarray(adj[:, lo : lo + rows])
        slab[lo + idx, idx] += 1.0  # self loops (a = adj + I)
        in_maps.append({"aslab": slab, "xat": xat, "wb": wb})
    return in_maps


def gather(results, n_cores=N_CORES):
    return np.ascontiguousarray(
        np.concatenate([np.asarray(r["outT"]).T for r in results[:n_cores]], axis=0)
    ).astype(np.float32)


def kernel(x, adj_matrix, W, b):
    nc = _get_nc()
    in_maps = host_prep(x, adj_matrix, W, b)
    res = run_bass_kernel_spmd(nc, in_maps, core_ids=list(range(N_CORES)))
    return gather(res.results)


if __name__ == "__main__":
    rng = np.random.default_rng(0)
    x = rng.standard_normal((N, D)).astype(np.float32)
    u = rng.random((N, N)).astype(np.float32)
    adj = (u < 0.01).astype(np.float32)
    adj = np.maximum(adj, adj.T) * (1.0 - np.eye(N, dtype=np.float32))
    W = rng.standard_normal((D, D)).astype(np.float32) / 8.0
    b = np.zeros(D, np.float32)
    out = kernel(x, adj, W, b)
    print(out.shape, out.dtype, float(np.abs(out).max()))
